# revision 10
# baseline (speedup 1.0000x reference)
"""Trainium2 Bass kernel: nn_MultiHeadCrossAttention (B=4, S=1024, H=1024, 16 heads).

Sharding: 8 cores = (batch b in 0..3) x (head-group g in 0..1, 8 heads each).
Per core: q/k/v projections for its head group on its batch, flash-style
attention in scores-transposed layout (softmax along the PSUM partition axis
via an augmented ones-column in the v matmul), and a partial out-projection.
Host sums the two per-batch partials and adds the output bias.

The bilinear span bias of the reference is constant along the softmax key
axis, so it cancels exactly in softmax and is not computed.

v2 changes vs the 193us baseline:
- all matmul streams in bf16 (same 1 cycle/row PE rate as f32r, half the
  DMA/SBUF traffic); psum accumulation stays fp32.
- softmax normalization via a [2,128]x[2,512] outer-product matmul that
  broadcasts the per-query reciprocals across partitions, replacing the
  DRAM-bounce 0-stride DMA dance (removes 48 small DMAs + bounce latency).
- software pipelining: the q-projection for the second query half is
  emitted one matmul per (hp,kt) slot inside the first attention half, and
  the out-projection of the first half inside the second, so the tensor
  engine has ready work while the scalar engine (exp, the attention-phase
  bottleneck at ~66us) runs.
"""
import os
import sys
import types

sys.path.insert(0, "/opt/trn_rl_repo")

# Optional NTFF profile hook shim (axon images lack antenv.axon_hooks).
if "antenv.axon_hooks" not in sys.modules:
    try:
        import trn_agent_boot.trn_boot as _tb

        _m = types.ModuleType("antenv.axon_hooks")
        _m.get_axon_ntff_profile_hook = (
            lambda: _tb._ntff_profile_via_ctypes("/opt/axon/libaxon_pjrt.so")
        )
        _m.set_axon_ntff_profile_hook = lambda h: None
        sys.modules["antenv.axon_hooks"] = _m
    except Exception:
        pass

import numpy as np
from ml_dtypes import bfloat16

import concourse.bass as bass
import concourse.mybir as mybir
import concourse.tile as tile
from concourse import bacc
from concourse.bass_utils import run_bass_kernel_spmd

F32 = mybir.dt.float32
F32R = mybir.dt.float32r
BF16 = mybir.dt.bfloat16
AF = mybir.ActivationFunctionType

B, S, H = 4, 1024, 1024
NHEADS = 16
HD = 64
G = 2                  # head groups (cores per batch)
NH = NHEADS // G       # 8 heads per core
F = NH * HD            # 512 per-core qkv features
HC = H // 128          # 8 contraction chunks for projections
KT = S // 128          # 8 key tiles
ST = S // 128          # 8 seq tiles
FC = F // 128          # 4 feature chunks
NQ = S // 512          # 2 query halves
SCALE = float(HD) ** -0.5

# Augmented v region per head pair, width 160:
#   cols 0..63    v_even
#   cols 64..95   1.0 (softmax denominator columns, shared, replicated so
#                 the sums land on a 32-aligned partition range)
#   cols 96..159  v_odd
# even head's ctx matmul uses cols [0:128]:  out p0-63=ctx_e, p64-95=sums_e
# odd  head's ctx matmul uses cols [32:160]: out p32-63=sums_o, p64-127=ctx_o
VREG = 160

_CACHE: dict = {}


def _build_nc():
    phase = os.environ.get("KBUILD_PHASE", "all")  # proj | attn | all
    nc = bacc.Bacc("TRN2", target_bir_lowering=False, debug=False)

    xT = nc.dram_tensor("xT", [H, S], BF16, kind="ExternalInput")    # aspect[b].T
    yT = nc.dram_tensor("yT", [H, S], BF16, kind="ExternalInput")    # opinion[b].T
    wqT = nc.dram_tensor("wqT", [H, F], BF16, kind="ExternalInput")  # Wq[g].T
    wkT = nc.dram_tensor("wkT", [H, F], BF16, kind="ExternalInput")
    wvT = nc.dram_tensor("wvT", [H, F], BF16, kind="ExternalInput")
    woT = nc.dram_tensor("woT", [F, H], BF16, kind="ExternalInput")  # Wo[:, g].T
    bqv = nc.dram_tensor("bqv", [F], F32, kind="ExternalInput")
    bkv = nc.dram_tensor("bkv", [F], F32, kind="ExternalInput")
    ebias = nc.dram_tensor("ebias", [S], F32, kind="ExternalInput")  # mask bias per key
    out = nc.dram_tensor("out", [S, H], F32, kind="ExternalOutput")

    with tile.TileContext(nc) as tc:
        const = tc.alloc_tile_pool(name="const", bufs=1)
        persist = tc.alloc_tile_pool(name="persist", bufs=1)

        bq_sb = const.tile([128, FC], F32, name="bq_sb")
        nc.sync.dma_start(out=bq_sb, in_=bqv.rearrange("(c p) -> p c", p=128))
        bk_sb = const.tile([128, FC], F32, name="bk_sb")
        nc.sync.dma_start(out=bk_sb, in_=bkv.rearrange("(c p) -> p c", p=128))
        eb_sb = const.tile([128, KT], F32, name="eb_sb")
        nc.sync.dma_start(out=eb_sb, in_=ebias.rearrange("(c p) -> p c", p=128))
        wo_sb = const.tile([128, FC, H], BF16, name="wo_sb")

        # selector for the reciprocal partition-broadcast outer product:
        # rbc[p, q] = sum_c sel[c, p] * srow[c, q]; row 0 carries recip_e
        # (-> partitions 0..63), row 32 carries recip_o (-> 64..127).
        sel = const.tile([64, 128], F32R, name="sel")
        sel_f = sel.bitcast(F32)
        nc.vector.memset(sel_f, 0.0)
        nc.vector.memset(sel_f[0:1, 0:64], 1.0)
        nc.vector.memset(sel_f[32:33, 64:128], 1.0)

        qT_sb = persist.tile([128, FC, S], BF16, name="qT_sb")
        kT_sb = persist.tile([128, FC, S], BF16, name="kT_sb")
        v_sb = persist.tile([128, KT, NH // 2, VREG], BF16, name="v_sb")

        # only the shared ones-columns [64:96] need init; the v columns are
        # fully written by the projection scatter.
        nc.vector.memset(
            v_sb.rearrange("p a b c -> p (a b) c")[:, :, 64:96], 1.0)

        psum = tc.alloc_tile_pool(name="psum", bufs=1, space="PSUM")

        # x/q weights live past the proj phase (q-proj of the second query
        # half is pipelined into the first attention half).
        qproj_sb = tc.alloc_tile_pool(name="qproj_sb", bufs=1)
        wq_sb = qproj_sb.tile([128, HC, F], BF16, name="wq_sb")
        xt_sb = qproj_sb.tile([128, HC, S], BF16, name="xt_sb")

        with tc.tile_pool(name="pjsb", bufs=1) as pjsb:
            wv_sb = pjsb.tile([128, HC, F], BF16, name="wv_sb")
            wk_sb = pjsb.tile([128, HC, F], BF16, name="wk_sb")
            yt_sb = pjsb.tile([128, HC, S], BF16, name="yt_sb")

            # DMA issue order matches compute order: v needs wv+yt, then k
            # needs wk, then q needs wq+xt; wo only at the end.
            for hc in range(HC):
                nc.sync.dma_start(out=wv_sb[:, hc, :],
                                  in_=wvT[hc * 128:(hc + 1) * 128, :])
                nc.sync.dma_start(out=yt_sb[:, hc, :],
                                  in_=yT[hc * 128:(hc + 1) * 128, :])
            for hc in range(HC):
                nc.sync.dma_start(out=wk_sb[:, hc, :],
                                  in_=wkT[hc * 128:(hc + 1) * 128, :])
            for hc in range(HC):
                nc.sync.dma_start(out=wq_sb[:, hc, :],
                                  in_=wqT[hc * 128:(hc + 1) * 128, :])
                nc.sync.dma_start(out=xt_sb[:, hc, :],
                                  in_=xT[hc * 128:(hc + 1) * 128, :])
            for fc in range(FC):
                nc.sync.dma_start(out=wo_sb[:, fc, :],
                                  in_=woT[fc * 128:(fc + 1) * 128, :])

            # v = opinion @ Wv.T : accumulate [s,128]x[128,F] over hc
            for st in range(ST):
                ps = psum.tile([128, F], F32, name="vps", tag="pp", bufs=4)
                for hc in range(HC):
                    nc.tensor.matmul(
                        ps,
                        yt_sb[:, hc, st * 128:(st + 1) * 128],
                        wv_sb[:, hc, :],
                        start=(hc == 0), stop=(hc == HC - 1),
                    )
                # scatter per head pair into the augmented v regions
                pv = ps.rearrange("p (hp e d) -> p hp e d", hp=NH // 2, e=2)
                nc.vector.tensor_copy(v_sb[:, st, :, 0:64], pv[:, :, 0, :])
                nc.vector.tensor_copy(v_sb[:, st, :, 96:160], pv[:, :, 1, :])

            # kT = (Wk @ opinion.T), both query halves; qT only nq=0 here
            for (w_sb, src_sb, b_sb, dst_sb, nqs) in (
                (wk_sb, yt_sb, bk_sb, kT_sb, (0, 1)),
                (wq_sb, xt_sb, bq_sb, qT_sb, (0,)),
            ):
                for fc in range(FC):
                    for nq in nqs:
                        ps = psum.tile([128, 512], F32, name="qkps", tag="pp", bufs=4)
                        for hc in range(HC):
                            nc.tensor.matmul(
                                ps,
                                w_sb[:, hc, fc * 128:(fc + 1) * 128],
                                src_sb[:, hc, nq * 512:(nq + 1) * 512],
                                start=(hc == 0), stop=(hc == HC - 1),
                            )
                        nc.vector.tensor_scalar_add(
                            dst_sb[:, fc, nq * 512:(nq + 1) * 512], ps,
                            b_sb[:, fc:fc + 1],
                        )

        if phase == "proj":
            with tc.tile_pool(name="dbg", bufs=2) as dbg:
                for fc in range(FC):
                    d1 = dbg.tile([128, S], F32, name="d1", tag="d")
                    nc.vector.tensor_copy(d1, qT_sb[:, fc, :])
                    nc.sync.dma_start(out=out[fc * 128:(fc + 1) * 128, :], in_=d1)
                    d2 = dbg.tile([128, S], F32, name="d2", tag="d")
                    nc.vector.tensor_copy(d2, kT_sb[:, fc, :])
                    nc.sync.dma_start(
                        out=out[512 + fc * 128:512 + (fc + 1) * 128, :], in_=d2)

        if phase in ("attn", "all"):
            attnsb = tc.alloc_tile_pool(name="attnsb", bufs=1)
            ctx_sb = attnsb.tile([128, FC, S], BF16, name="ctx_sb")

            with tc.tile_pool(name="exps", bufs=4) as exps, \
                 tc.tile_pool(name="outsb", bufs=3) as outsb, \
                 tc.tile_pool(name="smallp", bufs=2) as smallp:

                def emit_out_group(g):
                    # out-projection group g = (st, no) of the nq=0 rows:
                    # returns a filler callable emitting one matmul per call
                    st, no = g // 2, g % 2
                    ps = psum.tile([128, 512], F32, name="ops", tag="pp", bufs=4)

                    def step(fc2):
                        nc.tensor.matmul(
                            ps,
                            ctx_sb[:, fc2, st * 128:(st + 1) * 128],
                            wo_sb[:, fc2, no * 512:(no + 1) * 512],
                            start=(fc2 == 0), stop=(fc2 == FC - 1),
                        )
                        if fc2 == FC - 1:
                            ot = outsb.tile([128, 512], F32, name="ot", tag="ot")
                            nc.vector.tensor_copy(ot, ps)
                            nc.sync.dma_start(
                                out=out[st * 128:(st + 1) * 128,
                                        no * 512:(no + 1) * 512],
                                in_=ot)
                    return step

                for nq in range(NQ):
                    # fillers: ready tensor-engine work emitted between the
                    # exp-dependent ctx matmuls, one 512-col matmul per
                    # (hp, kt) slot (32 slots per nq half).
                    qp_state = {}

                    def filler_nq0(hp, kt):
                        # q-projection of the second query half, fc chunk=hp
                        if kt == 0:
                            qp_state["ps"] = psum.tile(
                                [128, 512], F32, name="qfps", tag="pp", bufs=4)
                        nc.tensor.matmul(
                            qp_state["ps"],
                            wq_sb[:, kt, hp * 128:(hp + 1) * 128],
                            xt_sb[:, kt, 512:1024],
                            start=(kt == 0), stop=(kt == KT - 1),
                        )
                        if kt == KT - 1:
                            nc.vector.tensor_scalar_add(
                                qT_sb[:, hp, 512:1024], qp_state["ps"],
                                bq_sb[:, hp:hp + 1],
                            )

                    og_state = {}

                    def filler_nq1(hp, kt):
                        # out-projection of the first query half: 8 groups
                        # of 4 accumulation steps over the 32 slots
                        slot = hp * KT + kt
                        g, fc2 = slot // FC, slot % FC
                        if fc2 == 0:
                            og_state["step"] = emit_out_group(g)
                        og_state["step"](fc2)

                    filler = (filler_nq0, filler_nq1)[nq] if phase == "all" else None

                    for hp in range(NH // 2):
                        fc = hp
                        cps_e = psum.tile([128, 512], F32, name="cps_e", tag="pp", bufs=4)
                        cps_o = psum.tile([128, 512], F32, name="cps_o", tag="pp", bufs=4)
                        for kt in range(KT):
                            sps = psum.tile([128, 2, 512], F32, name="sps", tag="sps", bufs=2)
                            for e in range(2):
                                p0 = 64 * e
                                # scoresT[k, q] = k_h . q_h over hd=64
                                nc.tensor.matmul(
                                    sps[:, e, :],
                                    kT_sb[p0:p0 + 64, fc, kt * 128:(kt + 1) * 128],
                                    qT_sb[p0:p0 + 64, fc, nq * 512:(nq + 1) * 512],
                                    start=True, stop=True,
                                )
                            ex = exps.tile([128, 2, 512], BF16, name="ex", tag="ex")
                            nc.scalar.activation(
                                ex, sps, AF.Exp,
                                bias=eb_sb[:, kt:kt + 1], scale=SCALE,
                            )
                            if filler is not None:
                                filler(hp, kt)
                            nc.tensor.matmul(
                                cps_e,
                                v_sb[:, kt, hp, 0:128],
                                ex[:, 0, :],
                                start=(kt == 0), stop=(kt == KT - 1),
                            )
                            nc.tensor.matmul(
                                cps_o,
                                v_sb[:, kt, hp, 32:160],
                                ex[:, 1, :],
                                start=(kt == 0), stop=(kt == KT - 1),
                            )
                        # normalize: exact DVE reciprocals of the (32x
                        # replicated) sums rows -- sums_e at p64:96 of cps_e,
                        # sums_o at p32:64 of cps_o -- then a small
                        # outer-product matmul broadcasts them across
                        # partitions: rbc[p,q] = recip_{e|o}[q].  All 64 srow
                        # partitions are written, so the zero rows of sel
                        # never multiply garbage.
                        srow = smallp.tile([64, 512], F32R, name="srow", tag="srow")
                        with nc.allow_low_precision(reason="f32r recip feeds broadcast matmul"):
                            nc.vector.reciprocal(out=srow[0:32, :], in_=cps_e[64:96, :])
                            nc.vector.reciprocal(out=srow[32:64, :], in_=cps_o[32:64, :])
                        rbc = psum.tile([128, 2, 512], F32, name="rbc", tag="sps", bufs=2)
                        nc.tensor.matmul(rbc[:, 0, :], sel, srow,
                                         start=True, stop=True)
                        rbs = smallp.tile([128, 512], F32, name="rbs", tag="rbs")
                        nc.vector.tensor_copy(rbs, rbc[:, 0, :])
                        nc.vector.tensor_mul(
                            ctx_sb[0:64, fc, nq * 512:(nq + 1) * 512],
                            cps_e[0:64, :], rbs[0:64, :])
                        nc.vector.tensor_mul(
                            ctx_sb[64:128, fc, nq * 512:(nq + 1) * 512],
                            cps_o[64:128, :], rbs[64:128, :])

                if phase == "all":
                    # out-projection for the second query half (tail)
                    og_state = {}
                    for slot in range(32):
                        g, fc2 = slot // FC, slot % FC
                        st, no = 4 + g // 2, g % 2
                        if fc2 == 0:
                            ps = psum.tile([128, 512], F32, name="ops", tag="pp", bufs=4)
                            og_state["ps"] = ps
                        ps = og_state["ps"]
                        nc.tensor.matmul(
                            ps,
                            ctx_sb[:, fc2, st * 128:(st + 1) * 128],
                            wo_sb[:, fc2, no * 512:(no + 1) * 512],
                            start=(fc2 == 0), stop=(fc2 == FC - 1),
                        )
                        if fc2 == FC - 1:
                            ot = outsb.tile([128, 512], F32, name="ot", tag="ot")
                            nc.vector.tensor_copy(ot, ps)
                            nc.sync.dma_start(
                                out=out[st * 128:(st + 1) * 128,
                                        no * 512:(no + 1) * 512],
                                in_=ot)

            if phase == "attn":
                with tc.tile_pool(name="dbg", bufs=2) as dbg:
                    for fc in range(FC):
                        d1 = dbg.tile([128, S], F32, name="d1", tag="d")
                        nc.vector.tensor_copy(d1, ctx_sb[:, fc, :])
                        nc.sync.dma_start(out=out[fc * 128:(fc + 1) * 128, :], in_=d1)
            attnsb.release()

        qproj_sb.release()
        psum.release()
        persist.release()
        const.release()

    nc.finalize()
    return nc


def get_nc():
    key = "nc:" + os.environ.get("KBUILD_PHASE", "all")
    if key not in _CACHE:
        _CACHE[key] = _build_nc()
    return _CACHE[key]


def make_in_maps(aspect_hidden, opinion_hidden, attention_mask,
                 Wq, bq, Wk, bk, Wv, bv, Wo, bo):
    asp = np.asarray(aspect_hidden, np.float32)
    opi = np.asarray(opinion_hidden, np.float32)
    mask = np.asarray(attention_mask)
    in_maps = []
    xTs = [np.ascontiguousarray(asp[b].T).astype(bfloat16) for b in range(B)]
    yTs = [np.ascontiguousarray(opi[b].T).astype(bfloat16) for b in range(B)]
    ebs = [np.where(mask[b] == 0, np.float32(-1e30), np.float32(0.0)).astype(np.float32)
           for b in range(B)]
    wqTs = [np.ascontiguousarray(Wq[g * F:(g + 1) * F, :].T).astype(bfloat16) for g in range(G)]
    wkTs = [np.ascontiguousarray(Wk[g * F:(g + 1) * F, :].T).astype(bfloat16) for g in range(G)]
    wvTs = [np.ascontiguousarray(Wv[g * F:(g + 1) * F, :].T).astype(bfloat16) for g in range(G)]
    woTs = [np.ascontiguousarray(Wo[:, g * F:(g + 1) * F].T).astype(bfloat16) for g in range(G)]
    bqs = [np.ascontiguousarray(bq[g * F:(g + 1) * F]) for g in range(G)]
    bks = [np.ascontiguousarray(bk[g * F:(g + 1) * F]) for g in range(G)]
    for c in range(8):
        b, g = c // G, c % G
        in_maps.append({
            "xT": xTs[b], "yT": yTs[b],
            "wqT": wqTs[g], "wkT": wkTs[g], "wvT": wvTs[g], "woT": woTs[g],
            "bqv": bqs[g], "bkv": bks[g], "ebias": ebs[b],
        })
    return in_maps


def kernel(aspect_hidden, opinion_hidden, attention_mask,
           Wq, bq, Wk, bk, Wv, bv, Wo, bo, Wbil, bbil):
    Wq = np.asarray(Wq, np.float32); bq = np.asarray(bq, np.float32)
    Wk = np.asarray(Wk, np.float32); bk = np.asarray(bk, np.float32)
    Wv = np.asarray(Wv, np.float32); bv = np.asarray(bv, np.float32)
    Wo = np.asarray(Wo, np.float32); bo = np.asarray(bo, np.float32)

    nc = get_nc()
    in_maps = make_in_maps(aspect_hidden, opinion_hidden, attention_mask,
                           Wq, bq, Wk, bk, Wv, bv, Wo, bo)
    trace = bool(int(os.environ.get("KERNEL_TRACE", "0")))
    res = run_bass_kernel_spmd(nc, in_maps, core_ids=list(range(8)), trace=trace)
    _CACHE["last_results"] = res

    # v-bias folds into a constant output offset: softmax rows sum to 1, so
    # ctx picks up +bv exactly, and out picks up +Wo @ bv.
    bo_eff = (bo.astype(np.float64) + Wo.astype(np.float64) @ bv.astype(np.float64))
    outs = np.empty((B, S, H), np.float32)
    for b in range(B):
        acc = (res.results[G * b]["out"].astype(np.float64)
               + res.results[G * b + 1]["out"].astype(np.float64) + bo_eff)
        outs[b] = acc.astype(np.float32)
    return outs


# revision 17
# speedup vs baseline: 1.3795x; 1.3795x over previous
"""Trainium2 Bass kernel: nn_MultiHeadCrossAttention (B=4, S=1024, H=1024, 16 heads).

Sharding: 8 cores = (batch b in 0..3) x (head-group g in 0..1, 8 heads each).
Per core: q/k/v projections for its head group on its batch, flash-style
attention in scores-transposed layout (softmax along the PSUM partition axis
via an augmented ones-column in the v matmul), and a partial out-projection.
Host sums the two per-batch partials and adds the output bias.

The bilinear span bias of the reference is constant along the softmax key
axis, so it cancels exactly in softmax and is not computed.

v2 changes vs the 193us baseline:
- all matmul streams in bf16 (same 1 cycle/row PE rate as f32r, half the
  DMA/SBUF traffic); psum accumulation stays fp32.
- softmax normalization via a [2,128]x[2,512] outer-product matmul that
  broadcasts the per-query reciprocals across partitions, replacing the
  DRAM-bounce 0-stride DMA dance (removes 48 small DMAs + bounce latency).
- software pipelining: the q-projection for the second query half is
  emitted one matmul per (hp,kt) slot inside the first attention half, and
  the out-projection of the first half inside the second, so the tensor
  engine has ready work while the scalar engine (exp, the attention-phase
  bottleneck at ~66us) runs.
"""
import os
import sys
import types

sys.path.insert(0, "/opt/trn_rl_repo")

# Optional NTFF profile hook shim (axon images lack antenv.axon_hooks).
if "antenv.axon_hooks" not in sys.modules:
    try:
        import trn_agent_boot.trn_boot as _tb

        _m = types.ModuleType("antenv.axon_hooks")
        _m.get_axon_ntff_profile_hook = (
            lambda: _tb._ntff_profile_via_ctypes("/opt/axon/libaxon_pjrt.so")
        )
        _m.set_axon_ntff_profile_hook = lambda h: None
        sys.modules["antenv.axon_hooks"] = _m
    except Exception:
        pass

import numpy as np
from ml_dtypes import bfloat16

import concourse.bass as bass
import concourse.mybir as mybir
import concourse.tile as tile
from concourse import bacc
from concourse.bass_utils import run_bass_kernel_spmd

F32 = mybir.dt.float32
F32R = mybir.dt.float32r
BF16 = mybir.dt.bfloat16
AF = mybir.ActivationFunctionType

B, S, H = 4, 1024, 1024
NHEADS = 16
HD = 64
G = 2                  # head groups (cores per batch)
NH = NHEADS // G       # 8 heads per core
F = NH * HD            # 512 per-core qkv features
HC = H // 128          # 8 contraction chunks for projections
KT = S // 128          # 8 key tiles
ST = S // 128          # 8 seq tiles
FC = F // 128          # 4 feature chunks
NQ = S // 512          # 2 query halves
SCALE = float(HD) ** -0.5

# Augmented v region per head pair, width 160:
#   cols 0..63    v_even
#   cols 64..95   1.0 (softmax denominator columns, shared, replicated so
#                 the sums land on a 32-aligned partition range)
#   cols 96..159  v_odd
# even head's ctx matmul uses cols [0:128]:  out p0-63=ctx_e, p64-95=sums_e
# odd  head's ctx matmul uses cols [32:160]: out p32-63=sums_o, p64-127=ctx_o
VREG = 160

_CACHE: dict = {}


def _build_nc():
    phase = os.environ.get("KBUILD_PHASE", "all")  # proj | attn | all
    nc = bacc.Bacc("TRN2", target_bir_lowering=False, debug=False)

    xT = nc.dram_tensor("xT", [H, S], BF16, kind="ExternalInput")    # aspect[b].T
    yT = nc.dram_tensor("yT", [H, S], BF16, kind="ExternalInput")    # opinion[b].T
    wqT = nc.dram_tensor("wqT", [H, F], BF16, kind="ExternalInput")  # Wq[g].T
    wkT = nc.dram_tensor("wkT", [H, F], BF16, kind="ExternalInput")
    wvT = nc.dram_tensor("wvT", [H, F], BF16, kind="ExternalInput")
    woT = nc.dram_tensor("woT", [F, H], BF16, kind="ExternalInput")  # Wo[:, g].T
    bqv = nc.dram_tensor("bqv", [F], F32, kind="ExternalInput")
    bkv = nc.dram_tensor("bkv", [F], F32, kind="ExternalInput")
    ebias = nc.dram_tensor("ebias", [S], F32, kind="ExternalInput")  # mask bias per key
    out = nc.dram_tensor("out", [S, H], F32, kind="ExternalOutput")
    # DRAM bounce for the softmax reciprocals (DRAM APs allow the 0-stride
    # partition-broadcast read that SBUF APs reject).
    rsc = nc.dram_tensor("rsc", [NH // 2 * NQ, 2, 512], F32)

    with tile.TileContext(nc) as tc:
        const = tc.alloc_tile_pool(name="const", bufs=1)
        persist = tc.alloc_tile_pool(name="persist", bufs=1)

        bq_sb = const.tile([128, FC], F32, name="bq_sb")
        nc.sync.dma_start(out=bq_sb, in_=bqv.rearrange("(c p) -> p c", p=128))
        bk_sb = const.tile([128, FC], F32, name="bk_sb")
        nc.sync.dma_start(out=bk_sb, in_=bkv.rearrange("(c p) -> p c", p=128))
        eb_sb = const.tile([128, KT], F32, name="eb_sb")
        nc.sync.dma_start(out=eb_sb, in_=ebias.rearrange("(c p) -> p c", p=128))
        wo_sb = const.tile([128, FC, H], BF16, name="wo_sb")



        qT_sb = persist.tile([128, FC, S], BF16, name="qT_sb")
        kT_sb = persist.tile([128, FC, S], BF16, name="kT_sb")
        v_sb = persist.tile([128, KT, NH // 2, VREG], BF16, name="v_sb")

        # only the shared ones-columns [64:96] need init; the v columns are
        # fully written by the projection scatter.
        nc.vector.memset(
            v_sb.rearrange("p a b c -> p (a b) c")[:, :, 64:96], 1.0)

        psum = tc.alloc_tile_pool(name="psum", bufs=1, space="PSUM")

        # x/q weights live past the proj phase (q-proj of the second query
        # half is pipelined into the first attention half).
        qproj_sb = tc.alloc_tile_pool(name="qproj_sb", bufs=1)
        wq_sb = qproj_sb.tile([128, HC, F], BF16, name="wq_sb")
        xt_sb = qproj_sb.tile([128, HC, S], BF16, name="xt_sb")

        with tc.tile_pool(name="pjsb", bufs=1) as pjsb:
            wv_sb = pjsb.tile([128, HC, F], BF16, name="wv_sb")
            wk_sb = pjsb.tile([128, HC, F], BF16, name="wk_sb")
            yt_sb = pjsb.tile([128, HC, S], BF16, name="yt_sb")

            # DMA issue order matches compute order: v needs wv+yt, then k
            # needs wk, then q needs wq+xt; wo only at the end.
            for hc in range(HC):
                nc.sync.dma_start(out=wv_sb[:, hc, :],
                                  in_=wvT[hc * 128:(hc + 1) * 128, :])
                nc.sync.dma_start(out=yt_sb[:, hc, :],
                                  in_=yT[hc * 128:(hc + 1) * 128, :])
            for hc in range(HC):
                nc.sync.dma_start(out=wk_sb[:, hc, :],
                                  in_=wkT[hc * 128:(hc + 1) * 128, :])
            for hc in range(HC):
                nc.sync.dma_start(out=wq_sb[:, hc, :],
                                  in_=wqT[hc * 128:(hc + 1) * 128, :])
                nc.sync.dma_start(out=xt_sb[:, hc, :],
                                  in_=xT[hc * 128:(hc + 1) * 128, :])
            for fc in range(FC):
                nc.sync.dma_start(out=wo_sb[:, fc, :],
                                  in_=woT[fc * 128:(fc + 1) * 128, :])

            # v = opinion @ Wv.T : accumulate [s,128]x[128,F] over hc
            for st in range(ST):
                ps = psum.tile([128, F], F32, name="vps", tag="pp", bufs=2)
                for hc in range(HC):
                    nc.tensor.matmul(
                        ps,
                        yt_sb[:, hc, st * 128:(st + 1) * 128],
                        wv_sb[:, hc, :],
                        start=(hc == 0), stop=(hc == HC - 1),
                    )
                # scatter per head pair into the augmented v regions
                pv = ps.rearrange("p (hp e d) -> p hp e d", hp=NH // 2, e=2)
                nc.vector.tensor_copy(v_sb[:, st, :, 0:64], pv[:, :, 0, :])
                nc.vector.tensor_copy(v_sb[:, st, :, 96:160], pv[:, :, 1, :])

            # kT = (Wk @ opinion.T), both query halves; qT only nq=0 here
            for (w_sb, src_sb, b_sb, dst_sb, nqs) in (
                (wk_sb, yt_sb, bk_sb, kT_sb, (0, 1)),
                (wq_sb, xt_sb, bq_sb, qT_sb, (0,)),
            ):
                for fc in range(FC):
                    for nq in nqs:
                        ps = psum.tile([128, 512], F32, name="qkps", tag="pp", bufs=2)
                        for hc in range(HC):
                            nc.tensor.matmul(
                                ps,
                                w_sb[:, hc, fc * 128:(fc + 1) * 128],
                                src_sb[:, hc, nq * 512:(nq + 1) * 512],
                                start=(hc == 0), stop=(hc == HC - 1),
                            )
                        nc.vector.tensor_scalar_add(
                            dst_sb[:, fc, nq * 512:(nq + 1) * 512], ps,
                            b_sb[:, fc:fc + 1],
                        )

        if phase == "proj":
            with tc.tile_pool(name="dbg", bufs=2) as dbg:
                for fc in range(FC):
                    d1 = dbg.tile([128, S], F32, name="d1", tag="d")
                    nc.vector.tensor_copy(d1, qT_sb[:, fc, :])
                    nc.sync.dma_start(out=out[fc * 128:(fc + 1) * 128, :], in_=d1)
                    d2 = dbg.tile([128, S], F32, name="d2", tag="d")
                    nc.vector.tensor_copy(d2, kT_sb[:, fc, :])
                    nc.sync.dma_start(
                        out=out[512 + fc * 128:512 + (fc + 1) * 128, :], in_=d2)

        if phase in ("attn", "all"):
            attnsb = tc.alloc_tile_pool(name="attnsb", bufs=1)
            ctx_sb = attnsb.tile([128, FC, S], BF16, name="ctx_sb")

            with tc.tile_pool(name="exps", bufs=4) as exps, \
                 tc.tile_pool(name="outsb", bufs=3) as outsb, \
                 tc.tile_pool(name="ubufs", bufs=2) as ubufs, \
                 tc.tile_pool(name="smallp", bufs=2) as smallp:

                def emit_out_group(g):
                    # out-projection group g = (st, no): returns a filler
                    # callable emitting one matmul per call
                    st, no = g // 2, g % 2
                    ps = psum.tile([128, 512], F32, name="ops", tag="pp", bufs=2)

                    def step(fc2):
                        nc.tensor.matmul(
                            ps,
                            ctx_sb[:, fc2, st * 128:(st + 1) * 128],
                            wo_sb[:, fc2, no * 512:(no + 1) * 512],
                            start=(fc2 == 0), stop=(fc2 == FC - 1),
                        )
                        if fc2 == FC - 1:
                            ot = outsb.tile([128, 512], F32, name="ot", tag="ot")
                            nc.vector.tensor_copy(ot, ps)
                            nc.sync.dma_start(
                                out=out[st * 128:(st + 1) * 128,
                                        no * 512:(no + 1) * 512],
                                in_=ot)
                    return step

                for nq in range(NQ):
                    # fillers: ready tensor-engine work emitted between the
                    # exp-dependent ctx matmuls, one 512-col matmul per
                    # (hp, kt) slot (32 slots per nq half).
                    qp_state = {}

                    def filler_nq0(hp, kt):
                        # q-projection of the second query half, fc chunk=hp
                        if kt == 0:
                            qp_state["ps"] = psum.tile(
                                [128, 512], F32, name="qfps", tag="pp", bufs=2)
                        nc.tensor.matmul(
                            qp_state["ps"],
                            wq_sb[:, kt, hp * 128:(hp + 1) * 128],
                            xt_sb[:, kt, 512:1024],
                            start=(kt == 0), stop=(kt == KT - 1),
                        )
                        if kt == KT - 1:
                            nc.vector.tensor_scalar_add(
                                qT_sb[:, hp, 512:1024], qp_state["ps"],
                                bq_sb[:, hp:hp + 1],
                            )

                    og_state = {}

                    def filler_nq1(hp, kt):
                        # out-projection of the first query half: 8 groups
                        # of 4 accumulation steps over the 32 slots
                        slot = hp * KT + kt
                        g, fc2 = slot // FC, slot % FC
                        if fc2 == 0:
                            og_state["step"] = emit_out_group(g)
                        og_state["step"](fc2)

                    filler = (filler_nq0, filler_nq1)[nq] if phase == "all" else None

                    for hp in range(NH // 2):
                        fc = hp
                        cps = psum.tile([128, 2, 512], F32, name="cps", tag="cps", bufs=1)
                        for kt in range(KT):
                            sps = psum.tile([128, 2, 512], F32, name="sps", tag="sps", bufs=2)
                            for e in range(2):
                                p0 = 64 * e
                                # scoresT[k, q] = k_h . q_h over hd=64
                                nc.tensor.matmul(
                                    sps[:, e, :],
                                    kT_sb[p0:p0 + 64, fc, kt * 128:(kt + 1) * 128],
                                    qT_sb[p0:p0 + 64, fc, nq * 512:(nq + 1) * 512],
                                    start=True, stop=True,
                                )
                            ex = exps.tile([128, 2, 512], BF16, name="ex", tag="ex")
                            nc.scalar.activation(
                                ex, sps, AF.Exp,
                                bias=eb_sb[:, kt:kt + 1], scale=SCALE,
                            )
                            if filler is not None:
                                filler(hp, kt)
                            nc.tensor.matmul(
                                cps[:, 0, :],
                                v_sb[:, kt, hp, 0:128],
                                ex[:, 0, :],
                                start=(kt == 0), stop=(kt == KT - 1),
                            )
                            nc.tensor.matmul(
                                cps[:, 1, :],
                                v_sb[:, kt, hp, 32:160],
                                ex[:, 1, :],
                                start=(kt == 0), stop=(kt == KT - 1),
                            )
                        # normalize.  One [128,2,512] copy drains both ctx
                        # accumulators to SBUF, freeing the psum bank pair
                        # after ~1.3us instead of holding it through the
                        # whole reciprocal chain; everything below is
                        # DVE/DMA-only (no psum), fully off the tensor
                        # engine's critical path.  The sums rows are
                        # repartitioned to [128,4] so the exact DVE
                        # reciprocal runs on a tiny free-size, then
                        # partition-broadcast via a DRAM bounce (DRAM APs
                        # allow the 0-stride read that SBUF APs reject).
                        ubuf = ubufs.tile([128, 2, 512], F32, name="ubuf", tag="ub")
                        nc.vector.tensor_copy(ubuf, cps)
                        sp = smallp.tile([128, 8], F32, name="sp", tag="sp")
                        nc.sync.dma_start(out=sp[:, 0:4], in_=ubuf[64:65, 0, :])
                        nc.sync.dma_start(out=sp[:, 4:8], in_=ubuf[32:33, 1, :])
                        rp = smallp.tile([128, 8], F32, name="rp", tag="rp")
                        nc.vector.reciprocal(out=rp, in_=sp)
                        it = hp * NQ + nq
                        nc.sync.dma_start(out=rsc[it, 0, :], in_=rp[:, 0:4])
                        nc.sync.dma_start(out=rsc[it, 1, :], in_=rp[:, 4:8])
                        rbc = smallp.tile([128, 512], F32, name="rbc", tag="rbc")
                        for e in range(2):
                            src = rsc[it, e, :]
                            nc.sync.dma_start(
                                out=rbc[64 * e:64 * e + 64, :],
                                in_=bass.AP(tensor=src.tensor, offset=src.offset,
                                            ap=[[0, 64]] + list(src.ap)))
                        nc.vector.tensor_mul(
                            ctx_sb[0:64, fc, nq * 512:(nq + 1) * 512],
                            ubuf[0:64, 0, :], rbc[0:64, :])
                        nc.vector.tensor_mul(
                            ctx_sb[64:128, fc, nq * 512:(nq + 1) * 512],
                            ubuf[64:128, 1, :], rbc[64:128, :])

                if phase == "all":
                    # out-projection for the second query half (tail)
                    og_state = {}
                    for slot in range(32):
                        g, fc2 = slot // FC, slot % FC
                        st, no = 4 + g // 2, g % 2
                        if fc2 == 0:
                            ps = psum.tile([128, 512], F32, name="ops", tag="pp", bufs=2)
                            og_state["ps"] = ps
                        ps = og_state["ps"]
                        nc.tensor.matmul(
                            ps,
                            ctx_sb[:, fc2, st * 128:(st + 1) * 128],
                            wo_sb[:, fc2, no * 512:(no + 1) * 512],
                            start=(fc2 == 0), stop=(fc2 == FC - 1),
                        )
                        if fc2 == FC - 1:
                            ot = outsb.tile([128, 512], F32, name="ot", tag="ot")
                            nc.vector.tensor_copy(ot, ps)
                            nc.sync.dma_start(
                                out=out[st * 128:(st + 1) * 128,
                                        no * 512:(no + 1) * 512],
                                in_=ot)

            if phase == "attn":
                with tc.tile_pool(name="dbg", bufs=2) as dbg:
                    for fc in range(FC):
                        d1 = dbg.tile([128, S], F32, name="d1", tag="d")
                        nc.vector.tensor_copy(d1, ctx_sb[:, fc, :])
                        nc.sync.dma_start(out=out[fc * 128:(fc + 1) * 128, :], in_=d1)
            attnsb.release()

        qproj_sb.release()
        psum.release()
        persist.release()
        const.release()

    nc.finalize()
    return nc


def get_nc():
    key = "nc:" + os.environ.get("KBUILD_PHASE", "all")
    if key not in _CACHE:
        _CACHE[key] = _build_nc()
    return _CACHE[key]


def make_in_maps(aspect_hidden, opinion_hidden, attention_mask,
                 Wq, bq, Wk, bk, Wv, bv, Wo, bo):
    asp = np.asarray(aspect_hidden, np.float32)
    opi = np.asarray(opinion_hidden, np.float32)
    mask = np.asarray(attention_mask)
    in_maps = []
    xTs = [np.ascontiguousarray(asp[b].T).astype(bfloat16) for b in range(B)]
    yTs = [np.ascontiguousarray(opi[b].T).astype(bfloat16) for b in range(B)]
    ebs = [np.where(mask[b] == 0, np.float32(-1e30), np.float32(0.0)).astype(np.float32)
           for b in range(B)]
    wqTs = [np.ascontiguousarray(Wq[g * F:(g + 1) * F, :].T).astype(bfloat16) for g in range(G)]
    wkTs = [np.ascontiguousarray(Wk[g * F:(g + 1) * F, :].T).astype(bfloat16) for g in range(G)]
    wvTs = [np.ascontiguousarray(Wv[g * F:(g + 1) * F, :].T).astype(bfloat16) for g in range(G)]
    woTs = [np.ascontiguousarray(Wo[:, g * F:(g + 1) * F].T).astype(bfloat16) for g in range(G)]
    bqs = [np.ascontiguousarray(bq[g * F:(g + 1) * F]) for g in range(G)]
    bks = [np.ascontiguousarray(bk[g * F:(g + 1) * F]) for g in range(G)]
    for c in range(8):
        b, g = c // G, c % G
        in_maps.append({
            "xT": xTs[b], "yT": yTs[b],
            "wqT": wqTs[g], "wkT": wkTs[g], "wvT": wvTs[g], "woT": woTs[g],
            "bqv": bqs[g], "bkv": bks[g], "ebias": ebs[b],
        })
    return in_maps


def kernel(aspect_hidden, opinion_hidden, attention_mask,
           Wq, bq, Wk, bk, Wv, bv, Wo, bo, Wbil, bbil):
    Wq = np.asarray(Wq, np.float32); bq = np.asarray(bq, np.float32)
    Wk = np.asarray(Wk, np.float32); bk = np.asarray(bk, np.float32)
    Wv = np.asarray(Wv, np.float32); bv = np.asarray(bv, np.float32)
    Wo = np.asarray(Wo, np.float32); bo = np.asarray(bo, np.float32)

    nc = get_nc()
    in_maps = make_in_maps(aspect_hidden, opinion_hidden, attention_mask,
                           Wq, bq, Wk, bk, Wv, bv, Wo, bo)
    trace = bool(int(os.environ.get("KERNEL_TRACE", "0")))
    res = run_bass_kernel_spmd(nc, in_maps, core_ids=list(range(8)), trace=trace)
    _CACHE["last_results"] = res

    # v-bias folds into a constant output offset: softmax rows sum to 1, so
    # ctx picks up +bv exactly, and out picks up +Wo @ bv.
    bo_eff = (bo.astype(np.float64) + Wo.astype(np.float64) @ bv.astype(np.float64))
    outs = np.empty((B, S, H), np.float32)
    for b in range(B):
        acc = (res.results[G * b]["out"].astype(np.float64)
               + res.results[G * b + 1]["out"].astype(np.float64) + bo_eff)
        outs[b] = acc.astype(np.float32)
    return outs


# revision 23
# speedup vs baseline: 1.4180x; 1.0279x over previous
"""Trainium2 Bass kernel: nn_MultiHeadCrossAttention (B=4, S=1024, H=1024, 16 heads).

Sharding: 8 cores = (batch b in 0..3) x (head-group g in 0..1, 8 heads each).
Per core: q/k/v projections for its head group on its batch, flash-style
attention in scores-transposed layout (softmax along the PSUM partition axis
via an augmented ones-column in the v matmul), and a partial out-projection.
Host sums the two per-batch partials and adds the output bias.

The bilinear span bias of the reference is constant along the softmax key
axis, so it cancels exactly in softmax and is not computed.

v2 changes vs the 193us baseline:
- all matmul streams in bf16 (same 1 cycle/row PE rate as f32r, half the
  DMA/SBUF traffic); psum accumulation stays fp32.
- softmax normalization via a [2,128]x[2,512] outer-product matmul that
  broadcasts the per-query reciprocals across partitions, replacing the
  DRAM-bounce 0-stride DMA dance (removes 48 small DMAs + bounce latency).
- software pipelining: the q-projection for the second query half is
  emitted one matmul per (hp,kt) slot inside the first attention half, and
  the out-projection of the first half inside the second, so the tensor
  engine has ready work while the scalar engine (exp, the attention-phase
  bottleneck at ~66us) runs.
"""
import os
import sys
import types

sys.path.insert(0, "/opt/trn_rl_repo")

# Optional NTFF profile hook shim (axon images lack antenv.axon_hooks).
if "antenv.axon_hooks" not in sys.modules:
    try:
        import trn_agent_boot.trn_boot as _tb

        _m = types.ModuleType("antenv.axon_hooks")
        _m.get_axon_ntff_profile_hook = (
            lambda: _tb._ntff_profile_via_ctypes("/opt/axon/libaxon_pjrt.so")
        )
        _m.set_axon_ntff_profile_hook = lambda h: None
        sys.modules["antenv.axon_hooks"] = _m
    except Exception:
        pass

import numpy as np
from ml_dtypes import bfloat16

import concourse.bass as bass
import concourse.mybir as mybir
import concourse.tile as tile
from concourse import bacc
from concourse.bass_utils import run_bass_kernel_spmd

F32 = mybir.dt.float32
F32R = mybir.dt.float32r
BF16 = mybir.dt.bfloat16
AF = mybir.ActivationFunctionType

B, S, H = 4, 1024, 1024
NHEADS = 16
HD = 64
G = 2                  # head groups (cores per batch)
NH = NHEADS // G       # 8 heads per core
F = NH * HD            # 512 per-core qkv features
HC = H // 128          # 8 contraction chunks for projections
KT = S // 128          # 8 key tiles
ST = S // 128          # 8 seq tiles
FC = F // 128          # 4 feature chunks
NQ = S // 512          # 2 query halves
SCALE = float(HD) ** -0.5

# Augmented v region per head pair, width 160:
#   cols 0..63    v_even
#   cols 64..95   1.0 (softmax denominator columns, shared, replicated so
#                 the sums land on a 32-aligned partition range)
#   cols 96..159  v_odd
# even head's ctx matmul uses cols [0:128]:  out p0-63=ctx_e, p64-95=sums_e
# odd  head's ctx matmul uses cols [32:160]: out p32-63=sums_o, p64-127=ctx_o
VREG = 160

_CACHE: dict = {}


def _build_nc():
    phase = os.environ.get("KBUILD_PHASE", "all")  # proj | attn | all
    nc = bacc.Bacc("TRN2", target_bir_lowering=False, debug=False)

    xT = nc.dram_tensor("xT", [H, S], BF16, kind="ExternalInput")    # aspect[b].T
    yT = nc.dram_tensor("yT", [H, S], BF16, kind="ExternalInput")    # opinion[b].T
    wqT = nc.dram_tensor("wqT", [H, F], BF16, kind="ExternalInput")  # Wq[g].T
    wkT = nc.dram_tensor("wkT", [H, F], BF16, kind="ExternalInput")
    wvT = nc.dram_tensor("wvT", [H, F], BF16, kind="ExternalInput")
    woT = nc.dram_tensor("woT", [F, H], BF16, kind="ExternalInput")  # Wo[:, g].T
    bqv = nc.dram_tensor("bqv", [F], F32, kind="ExternalInput")
    bkv = nc.dram_tensor("bkv", [F], F32, kind="ExternalInput")
    ebias = nc.dram_tensor("ebias", [S], F32, kind="ExternalInput")  # mask bias per key
    out = nc.dram_tensor("out", [S, H], BF16, kind="ExternalOutput")
    # DRAM bounce for the softmax reciprocals (DRAM APs allow the 0-stride
    # partition-broadcast read that SBUF APs reject).
    rsc = nc.dram_tensor("rsc", [NH // 2 * NQ, 2, 512], F32)

    with tile.TileContext(nc) as tc:
        const = tc.alloc_tile_pool(name="const", bufs=1)
        persist = tc.alloc_tile_pool(name="persist", bufs=1)

        bq_sb = const.tile([128, FC], F32, name="bq_sb")
        nc.sync.dma_start(out=bq_sb, in_=bqv.rearrange("(c p) -> p c", p=128))
        bk_sb = const.tile([128, FC], F32, name="bk_sb")
        nc.sync.dma_start(out=bk_sb, in_=bkv.rearrange("(c p) -> p c", p=128))
        eb_sb = const.tile([128, KT], F32, name="eb_sb")
        nc.sync.dma_start(out=eb_sb, in_=ebias.rearrange("(c p) -> p c", p=128))
        wo_sb = const.tile([128, FC, H], BF16, name="wo_sb")



        qT_sb = persist.tile([128, FC, S], BF16, name="qT_sb")
        kT_sb = persist.tile([128, FC, S], BF16, name="kT_sb")
        v_sb = persist.tile([128, KT, NH // 2, VREG], BF16, name="v_sb")

        # only the shared ones-columns [64:96] need init; the v columns are
        # fully written by the projection scatter.
        nc.vector.memset(
            v_sb.rearrange("p a b c -> p (a b) c")[:, :, 64:96], 1.0)

        psum = tc.alloc_tile_pool(name="psum", bufs=1, space="PSUM")

        # x/q weights live past the proj phase (q-proj of the second query
        # half is pipelined into the first attention half).
        qproj_sb = tc.alloc_tile_pool(name="qproj_sb", bufs=1)
        wq_sb = qproj_sb.tile([128, HC, F], BF16, name="wq_sb")
        xt_sb = qproj_sb.tile([128, HC, S], BF16, name="xt_sb")

        with tc.tile_pool(name="pjsb", bufs=1) as pjsb:
            wv_sb = pjsb.tile([128, HC, F], BF16, name="wv_sb")
            wk_sb = pjsb.tile([128, HC, F], BF16, name="wk_sb")
            yt_sb = pjsb.tile([128, HC, S], BF16, name="yt_sb")

            # DMA issue order matches compute order: v needs wv+yt, then k
            # needs wk, then q needs wq+xt; wo only at the end.
            for hc in range(HC):
                nc.sync.dma_start(out=wv_sb[:, hc, :],
                                  in_=wvT[hc * 128:(hc + 1) * 128, :])
                nc.sync.dma_start(out=yt_sb[:, hc, :],
                                  in_=yT[hc * 128:(hc + 1) * 128, :])
            for hc in range(HC):
                nc.sync.dma_start(out=wk_sb[:, hc, :],
                                  in_=wkT[hc * 128:(hc + 1) * 128, :])
            for hc in range(HC):
                nc.sync.dma_start(out=wq_sb[:, hc, :],
                                  in_=wqT[hc * 128:(hc + 1) * 128, :])
                nc.sync.dma_start(out=xt_sb[:, hc, :],
                                  in_=xT[hc * 128:(hc + 1) * 128, :])
            for fc in range(FC):
                nc.sync.dma_start(out=wo_sb[:, fc, :],
                                  in_=woT[fc * 128:(fc + 1) * 128, :])

            # v = opinion @ Wv.T : accumulate [s,128]x[128,F] over hc.
            # hc-major over 4 concurrent accumulation groups so the tensor
            # engine chases the arriving yt/wv DMA chunks instead of
            # waiting for the full contraction to land.
            for half in range(2):
                g01 = psum.tile([128, 2, 512], F32, name="vps01", tag="cps", bufs=1)
                g2 = psum.tile([128, F], F32, name="vps2", tag="pp", bufs=2)
                g3 = psum.tile([128, F], F32, name="vps3", tag="pp", bufs=2)
                grps = (g01[:, 0, :], g01[:, 1, :], g2, g3)
                for hc in range(HC):
                    for sg in range(4):
                        st = half * 4 + sg
                        nc.tensor.matmul(
                            grps[sg],
                            yt_sb[:, hc, st * 128:(st + 1) * 128],
                            wv_sb[:, hc, :],
                            start=(hc == 0), stop=(hc == HC - 1),
                        )
                for sg in range(4):
                    st = half * 4 + sg
                    # scatter per head pair into the augmented v regions
                    pv = grps[sg].rearrange("p (hp e d) -> p hp e d",
                                            hp=NH // 2, e=2)
                    nc.vector.tensor_copy(v_sb[:, st, :, 0:64], pv[:, :, 0, :])
                    nc.vector.tensor_copy(v_sb[:, st, :, 96:160], pv[:, :, 1, :])

            # kT = (Wk @ opinion.T), both query halves; qT only nq=0 here
            for (w_sb, src_sb, b_sb, dst_sb, nqs) in (
                (wk_sb, yt_sb, bk_sb, kT_sb, (0, 1)),
                (wq_sb, xt_sb, bq_sb, qT_sb, (0,)),
            ):
                for fc in range(FC):
                    for nq in nqs:
                        ps = psum.tile([128, 512], F32, name="qkps", tag="pp", bufs=2)
                        for hc in range(HC):
                            nc.tensor.matmul(
                                ps,
                                w_sb[:, hc, fc * 128:(fc + 1) * 128],
                                src_sb[:, hc, nq * 512:(nq + 1) * 512],
                                start=(hc == 0), stop=(hc == HC - 1),
                            )
                        nc.vector.tensor_scalar_add(
                            dst_sb[:, fc, nq * 512:(nq + 1) * 512], ps,
                            b_sb[:, fc:fc + 1],
                        )

        if phase == "proj":
            with tc.tile_pool(name="dbg", bufs=2) as dbg:
                for fc in range(FC):
                    d1 = dbg.tile([128, S], BF16, name="d1", tag="d")
                    nc.vector.tensor_copy(d1, qT_sb[:, fc, :])
                    nc.sync.dma_start(out=out[fc * 128:(fc + 1) * 128, :], in_=d1)
                    d2 = dbg.tile([128, S], BF16, name="d2", tag="d")
                    nc.vector.tensor_copy(d2, kT_sb[:, fc, :])
                    nc.sync.dma_start(
                        out=out[512 + fc * 128:512 + (fc + 1) * 128, :], in_=d2)

        if phase in ("attn", "all"):
            attnsb = tc.alloc_tile_pool(name="attnsb", bufs=1)
            ctx_sb = attnsb.tile([128, FC, S], BF16, name="ctx_sb")

            with tc.tile_pool(name="exps", bufs=4) as exps, \
                 tc.tile_pool(name="outsb", bufs=3) as outsb, \
                 tc.tile_pool(name="ubufs", bufs=2) as ubufs, \
                 tc.tile_pool(name="smallp", bufs=2) as smallp:

                def emit_out_group(g):
                    # out-projection group g = (st, no): returns a filler
                    # callable emitting one matmul per call
                    st, no = g // 2, g % 2
                    ps = psum.tile([128, 512], F32, name="ops", tag="pp", bufs=2)

                    def step(fc2):
                        nc.tensor.matmul(
                            ps,
                            ctx_sb[:, fc2, st * 128:(st + 1) * 128],
                            wo_sb[:, fc2, no * 512:(no + 1) * 512],
                            start=(fc2 == 0), stop=(fc2 == FC - 1),
                        )
                        if fc2 == FC - 1:
                            ot = outsb.tile([128, 512], BF16, name="ot", tag="ot")
                            nc.vector.tensor_copy(ot, ps)
                            nc.sync.dma_start(
                                out=out[st * 128:(st + 1) * 128,
                                        no * 512:(no + 1) * 512],
                                in_=ot)
                    return step

                for nq in range(NQ):
                    # fillers: ready tensor-engine work emitted between the
                    # exp-dependent ctx matmuls, one 512-col matmul per
                    # (hp, kt) slot (32 slots per nq half).
                    qp_state = {}

                    def filler_nq0(hp, kt):
                        # q-projection of the second query half, fc chunk=hp
                        if kt == 0:
                            qp_state["ps"] = psum.tile(
                                [128, 512], F32, name="qfps", tag="pp", bufs=2)
                        nc.tensor.matmul(
                            qp_state["ps"],
                            wq_sb[:, kt, hp * 128:(hp + 1) * 128],
                            xt_sb[:, kt, 512:1024],
                            start=(kt == 0), stop=(kt == KT - 1),
                        )
                        if kt == KT - 1:
                            nc.vector.tensor_scalar_add(
                                qT_sb[:, hp, 512:1024], qp_state["ps"],
                                bq_sb[:, hp:hp + 1],
                            )

                    og_state = {}

                    def filler_nq1(hp, kt):
                        # out-projection of the first query half: 8 groups
                        # of 4 accumulation steps over the 32 slots
                        slot = hp * KT + kt
                        g, fc2 = slot // FC, slot % FC
                        if fc2 == 0:
                            og_state["step"] = emit_out_group(g)
                        og_state["step"](fc2)

                    filler = (filler_nq0, filler_nq1)[nq] if phase == "all" else None

                    for hp in range(NH // 2):
                        fc = hp
                        cps = psum.tile([128, 2, 512], F32, name="cps", tag="cps", bufs=1)
                        for kt in range(KT):
                            sps = psum.tile([128, 2, 512], F32, name="sps", tag="sps", bufs=2)
                            for e in range(2):
                                p0 = 64 * e
                                # scoresT[k, q] = k_h . q_h over hd=64
                                nc.tensor.matmul(
                                    sps[:, e, :],
                                    kT_sb[p0:p0 + 64, fc, kt * 128:(kt + 1) * 128],
                                    qT_sb[p0:p0 + 64, fc, nq * 512:(nq + 1) * 512],
                                    start=True, stop=True,
                                )
                            ex = exps.tile([128, 2, 512], BF16, name="ex", tag="ex")
                            nc.scalar.activation(
                                ex, sps, AF.Exp,
                                bias=eb_sb[:, kt:kt + 1], scale=SCALE,
                            )
                            if filler is not None:
                                filler(hp, kt)
                            nc.tensor.matmul(
                                cps[:, 0, :],
                                v_sb[:, kt, hp, 0:128],
                                ex[:, 0, :],
                                start=(kt == 0), stop=(kt == KT - 1),
                            )
                            nc.tensor.matmul(
                                cps[:, 1, :],
                                v_sb[:, kt, hp, 32:160],
                                ex[:, 1, :],
                                start=(kt == 0), stop=(kt == KT - 1),
                            )
                        # normalize.  One [128,2,512] copy drains both ctx
                        # accumulators to SBUF, freeing the psum bank pair
                        # after ~1.3us instead of holding it through the
                        # whole reciprocal chain; everything below is
                        # DVE/DMA-only (no psum), fully off the tensor
                        # engine's critical path.  The sums rows are
                        # repartitioned to [128,4] so the exact DVE
                        # reciprocal runs on a tiny free-size, then
                        # partition-broadcast via a DRAM bounce (DRAM APs
                        # allow the 0-stride read that SBUF APs reject).
                        ubuf = ubufs.tile([128, 2, 512], F32, name="ubuf", tag="ub")
                        if nq == NQ - 1 and hp == NH // 2 - 1:
                            # last head pair: ACT has just gone idle; drain
                            # the accumulators there so the DVE queue doesn't
                            # delay the tail's dependency chain.
                            nc.scalar.copy(ubuf, cps)
                        else:
                            nc.vector.tensor_copy(ubuf, cps)
                        sp = smallp.tile([128, 8], F32, name="sp", tag="sp")
                        nc.sync.dma_start(out=sp[:, 0:4], in_=ubuf[64:65, 0, :])
                        nc.sync.dma_start(out=sp[:, 4:8], in_=ubuf[32:33, 1, :])
                        rp = smallp.tile([128, 8], F32, name="rp", tag="rp")
                        nc.vector.reciprocal(out=rp, in_=sp)
                        it = hp * NQ + nq
                        nc.sync.dma_start(out=rsc[it, 0, :], in_=rp[:, 0:4])
                        nc.sync.dma_start(out=rsc[it, 1, :], in_=rp[:, 4:8])
                        rbc = smallp.tile([128, 512], F32, name="rbc", tag="rbc")
                        for e in range(2):
                            src = rsc[it, e, :]
                            nc.sync.dma_start(
                                out=rbc[64 * e:64 * e + 64, :],
                                in_=bass.AP(tensor=src.tensor, offset=src.offset,
                                            ap=[[0, 64]] + list(src.ap)))
                        nc.vector.tensor_mul(
                            ctx_sb[0:64, fc, nq * 512:(nq + 1) * 512],
                            ubuf[0:64, 0, :], rbc[0:64, :])
                        nc.vector.tensor_mul(
                            ctx_sb[64:128, fc, nq * 512:(nq + 1) * 512],
                            ubuf[64:128, 1, :], rbc[64:128, :])

                if phase == "all":
                    # out-projection for the second query half (tail).
                    # psum->sbuf drains alternate between DVE and the (now
                    # idle) scalar engine so the copies pipeline 2-wide.
                    og_state = {}
                    for slot in range(32):
                        g, fc2 = slot // FC, slot % FC
                        st, no = 4 + g // 2, g % 2
                        if fc2 == 0:
                            ps = psum.tile([128, 512], F32, name="ops", tag="pp", bufs=2)
                            og_state["ps"] = ps
                        ps = og_state["ps"]
                        nc.tensor.matmul(
                            ps,
                            ctx_sb[:, fc2, st * 128:(st + 1) * 128],
                            wo_sb[:, fc2, no * 512:(no + 1) * 512],
                            start=(fc2 == 0), stop=(fc2 == FC - 1),
                        )
                        if fc2 == FC - 1:
                            ot = outsb.tile([128, 512], BF16, name="ot", tag="ot")
                            if g % 2 == 0:
                                nc.vector.tensor_copy(ot, ps)
                            else:
                                nc.scalar.copy(ot, ps)
                            nc.sync.dma_start(
                                out=out[st * 128:(st + 1) * 128,
                                        no * 512:(no + 1) * 512],
                                in_=ot)

            if phase == "attn":
                with tc.tile_pool(name="dbg", bufs=2) as dbg:
                    for fc in range(FC):
                        d1 = dbg.tile([128, S], BF16, name="d1", tag="d")
                        nc.vector.tensor_copy(d1, ctx_sb[:, fc, :])
                        nc.sync.dma_start(out=out[fc * 128:(fc + 1) * 128, :], in_=d1)
            attnsb.release()

        qproj_sb.release()
        psum.release()
        persist.release()
        const.release()

    nc.finalize()
    return nc


def get_nc():
    key = "nc:" + os.environ.get("KBUILD_PHASE", "all")
    if key not in _CACHE:
        _CACHE[key] = _build_nc()
    return _CACHE[key]


def make_in_maps(aspect_hidden, opinion_hidden, attention_mask,
                 Wq, bq, Wk, bk, Wv, bv, Wo, bo):
    asp = np.asarray(aspect_hidden, np.float32)
    opi = np.asarray(opinion_hidden, np.float32)
    mask = np.asarray(attention_mask)
    in_maps = []
    xTs = [np.ascontiguousarray(asp[b].T).astype(bfloat16) for b in range(B)]
    yTs = [np.ascontiguousarray(opi[b].T).astype(bfloat16) for b in range(B)]
    ebs = [np.where(mask[b] == 0, np.float32(-1e30), np.float32(0.0)).astype(np.float32)
           for b in range(B)]
    wqTs = [np.ascontiguousarray(Wq[g * F:(g + 1) * F, :].T).astype(bfloat16) for g in range(G)]
    wkTs = [np.ascontiguousarray(Wk[g * F:(g + 1) * F, :].T).astype(bfloat16) for g in range(G)]
    wvTs = [np.ascontiguousarray(Wv[g * F:(g + 1) * F, :].T).astype(bfloat16) for g in range(G)]
    woTs = [np.ascontiguousarray(Wo[:, g * F:(g + 1) * F].T).astype(bfloat16) for g in range(G)]
    bqs = [np.ascontiguousarray(bq[g * F:(g + 1) * F]) for g in range(G)]
    bks = [np.ascontiguousarray(bk[g * F:(g + 1) * F]) for g in range(G)]
    for c in range(8):
        b, g = c // G, c % G
        in_maps.append({
            "xT": xTs[b], "yT": yTs[b],
            "wqT": wqTs[g], "wkT": wkTs[g], "wvT": wvTs[g], "woT": woTs[g],
            "bqv": bqs[g], "bkv": bks[g], "ebias": ebs[b],
        })
    return in_maps


def kernel(aspect_hidden, opinion_hidden, attention_mask,
           Wq, bq, Wk, bk, Wv, bv, Wo, bo, Wbil, bbil):
    Wq = np.asarray(Wq, np.float32); bq = np.asarray(bq, np.float32)
    Wk = np.asarray(Wk, np.float32); bk = np.asarray(bk, np.float32)
    Wv = np.asarray(Wv, np.float32); bv = np.asarray(bv, np.float32)
    Wo = np.asarray(Wo, np.float32); bo = np.asarray(bo, np.float32)

    nc = get_nc()
    in_maps = make_in_maps(aspect_hidden, opinion_hidden, attention_mask,
                           Wq, bq, Wk, bk, Wv, bv, Wo, bo)
    trace = bool(int(os.environ.get("KERNEL_TRACE", "0")))
    res = run_bass_kernel_spmd(nc, in_maps, core_ids=list(range(8)), trace=trace)
    _CACHE["last_results"] = res

    # v-bias folds into a constant output offset: softmax rows sum to 1, so
    # ctx picks up +bv exactly, and out picks up +Wo @ bv.
    bo_eff = (bo.astype(np.float64) + Wo.astype(np.float64) @ bv.astype(np.float64))
    outs = np.empty((B, S, H), np.float32)
    for b in range(B):
        acc = (res.results[G * b]["out"].astype(np.float64)
               + res.results[G * b + 1]["out"].astype(np.float64) + bo_eff)
        outs[b] = acc.astype(np.float32)
    return outs
